# revision 25
# baseline (speedup 1.0000x reference)
"""DeepseekV2 MoE layer on 8 TRN2 NeuronCores (expert-parallel), v2.

Sharding: w1/w2 4-experts-per-core; router + token activations replicated;
shared expert tensor-parallel along FS (352/core, padded to 384). Routing
(softmax + grouped top-k) on device. Per core: gather each local expert's
<=256 tokens, MLP in bf16, weighted one-hot combine via matmul; plus the
shared-expert slice. Per-H-column-group partial [T, 512] blocks go through
4 pipelined ReduceScatters; each core emits rows [128k, 128(k+1)).

v2 changes vs baseline: all weights host-pre-tiled partition-major so every
DMA is ~1MB with >=1KB descriptors; x pre-cast to bf16 on host; all
non-router matmuls bf16 (combine was fp32r); DMA triggers on sync/scalar
queues (HWDGE) instead of gpsimd; expert prep (gather/transpose) front-
loaded; final combine accumulates shared+4 experts in PSUM (no SBUF acc);
collective split 4x and overlapped with compute.
"""

import numpy as np
import ml_dtypes

import concourse.bass as bass
import concourse.tile as tile
from concourse import bacc, mybir
from concourse.masks import make_identity

# problem shape
T, H = 1024, 2048
E, F = 32, 1408
F2 = 2 * F                      # 2816
G_GRP, TOPK_G, TOPK = 8, 3, 6
FS = 2 * F                      # 2816 shared intermediate
SCALE = 16.0
NCORES = 8
EL = E // NCORES                # 4 experts per core
C = 256                         # per-expert token capacity (max seen ~214)
P = 128
TT = T // P                     # 8 token tiles
HC = H // P                     # 16 h chunks
FT = F // P                     # 11 f tiles
F2T = F2 // P                   # 22
SSH = 384                       # padded shared shard (352 real)
SST = SSH // P                  # 3 shared m tiles

F32 = mybir.dt.float32
BF16 = mybir.dt.bfloat16
I32 = mybir.dt.int32
AF = mybir.ActivationFunctionType
OP = mybir.AluOpType

def build_program():
    nc = bacc.Bacc("TRN2", target_bir_lowering=False, debug=False,
                   num_devices=NCORES)

    xt32_d = nc.dram_tensor("xt32", [P, HC, T], F32, kind="ExternalInput")
    xtbf_d = nc.dram_tensor("xtbf", [P, HC, T], BF16, kind="ExternalInput")
    xrow_d = nc.dram_tensor("xrow", [T, H], BF16, kind="ExternalInput")
    wg_d = nc.dram_tensor("wg", [P, HC, E], F32, kind="ExternalInput")
    # w1 pair-major: [p, e*11+pair, k, 256] = w1[e, k*128+p, pair*256 + f']
    w1_d = nc.dram_tensor("w1l", [P, EL * FT, HC, 2 * P], BF16,
                          kind="ExternalInput")
    w2_d = nc.dram_tensor("w2l", [P, EL, FT, H], BF16, kind="ExternalInput")
    ws1_d = nc.dram_tensor("ws1l", [P, HC, 2 * SSH], BF16, kind="ExternalInput")
    ws2_d = nc.dram_tensor("ws2l", [P, SST, H], BF16, kind="ExternalInput")
    sel_d = nc.dram_tensor("sel", [E, EL], BF16, kind="ExternalInput")
    out_d = nc.dram_tensor("out", [P, H], BF16, kind="ExternalOutput")

    acc_ds = [nc.dram_tensor(f"acc{n}", [T, 512], BF16) for n in range(4)]
    rs_ds = [nc.dram_tensor(f"rs{n}", [P, 512], BF16) for n in range(4)]

    with tile.TileContext(nc) as tc:
        _build(nc, tc, locals())

    nc.compile()
    return nc


def _build(nc, tc, g):
    xt32_d, xtbf_d, xrow_d, wg_d = g["xt32_d"], g["xtbf_d"], g["xrow_d"], g["wg_d"]
    w1_d, w2_d, ws1_d, ws2_d = g["w1_d"], g["w2_d"], g["ws1_d"], g["ws2_d"]
    sel_d, out_d, acc_ds, rs_ds = g["sel_d"], g["out_d"], g["acc_ds"], g["rs_ds"]

    import contextlib
    ctx = contextlib.ExitStack()
    sb = ctx.enter_context(tc.tile_pool(name="sb", bufs=1))
    sb_rt = ctx.enter_context(tc.tile_pool(name="sb_rt", bufs=1))
    sb_sm = ctx.enter_context(tc.tile_pool(name="sb_sm", bufs=2))
    sb_xt = ctx.enter_context(tc.tile_pool(name="sb_xt", bufs=2))
    sb_xbf = ctx.enter_context(tc.tile_pool(name="sb_xbf", bufs=3))
    sb_ws1 = ctx.enter_context(tc.tile_pool(name="sb_ws1", bufs=2))
    sb_w1 = ctx.enter_context(tc.tile_pool(name="sb_w1", bufs=2))
    sb_w2 = ctx.enter_context(tc.tile_pool(name="sb_w2", bufs=3))
    sb_xe = ctx.enter_context(tc.tile_pool(name="sb_xe", bufs=2))
    sb_xet = ctx.enter_context(tc.tile_pool(name="sb_xet", bufs=2))
    sb_act = ctx.enter_context(tc.tile_pool(name="sb_act", bufs=4))
    sb_yn = ctx.enter_context(tc.tile_pool(name="sb_yn", bufs=2))
    sb_st = ctx.enter_context(tc.tile_pool(name="sb_st", bufs=2))
    ps_s = ctx.enter_context(tc.tile_pool(name="ps_s", bufs=6, space="PSUM"))
    ps_r = ctx.enter_context(tc.tile_pool(name="ps_r", bufs=2, space="PSUM"))

    # ---- constants ----
    ident = sb.tile([P, P], F32)
    make_identity(nc, ident[:])
    identb = sb.tile([P, P], BF16)
    nc.vector.tensor_copy(identb[:], ident[:])
    iota_c_row_i = sb.tile([P, C], I32)
    nc.gpsimd.iota(iota_c_row_i[:], pattern=[[1, C]], base=0, channel_multiplier=0)
    iota_c_row = sb.tile([P, C], F32)
    nc.vector.tensor_copy(iota_c_row[:], iota_c_row_i[:])
    iota_half_i = sb.tile([P, 2], I32)   # col h: value 128*h + p
    nc.gpsimd.iota(iota_half_i[:], pattern=[[P, 2]], base=0, channel_multiplier=1)
    iota_half = sb.tile([P, 2], BF16)
    nc.vector.tensor_copy(iota_half[:], iota_half_i[:])
    tok_iota_i = sb.tile([P, TT], I32)   # col k: value 128*k + p
    nc.gpsimd.iota(tok_iota_i[:], pattern=[[P, TT]], base=0, channel_multiplier=1)
    tok_iota = sb.tile([P, TT], F32)
    nc.vector.tensor_copy(tok_iota[:], tok_iota_i[:])
    ones_bf = sb.tile([P, T // 2], BF16)
    nc.vector.memset(ones_bf[:], 1.0)

    wg_sb = sb.tile([P, HC * E], F32)
    nc.scalar.dma_start(out=wg_sb[:], in_=wg_d[:, :, :])
    sel_sb = sb.tile([E, EL], BF16)
    nc.scalar.dma_start(out=sel_sb[:], in_=sel_d[:, :])

    # ---- router: logT[e, t] = wg.T @ x, fp32 (topk must match fp32 order) ----
    logT = sb_rt.tile([E, T], F32)
    psl = [ps_r.tile([P, T // 2], F32, tag="psr", name=f"psl{n}") for n in range(2)]
    for k in range(HC):
        xt = sb_xt.tile([P, T], F32, tag="xt")
        nc.sync.dma_start(out=xt[:], in_=xt32_d[:, k, :])
        for n in range(2):
            nc.tensor.matmul(psl[n][:E, :], wg_sb[:, k * E:(k + 1) * E],
                             xt[:, n * 512:(n + 1) * 512],
                             start=(k == 0), stop=(k == HC - 1))
    for n in range(2):
        nc.vector.tensor_copy(logT[:, n * 512:(n + 1) * 512], psl[n][:E, :])

    # ---- routing math on [P, TT*E] ----
    scores = sb_rt.tile([P, TT * E], F32)
    for k in range(TT):
        pst = ps_r.tile([P, 512], F32, tag="psr", name=f"tr{k}")
        nc.tensor.transpose(pst[:, :E], logT[:, k * P:(k + 1) * P], ident[:E, :E])
        nc.vector.tensor_copy(scores[:, k * E:(k + 1) * E], pst[:, :E])

    tmp8 = sb_rt.tile([P, 8], F32)
    for k in range(TT):
        blk = scores[:, k * E:(k + 1) * E]
        mx = sb_sm.tile([P, 1], F32, tag="rmax", name=f"rmax{k}")
        nc.vector.tensor_reduce(mx[:], blk, axis=mybir.AxisListType.X,
                                op=OP.max, negate=True)
        sm = sb_sm.tile([P, 1], F32, tag="rsum", name=f"rsum{k}")
        nc.scalar.activation(blk, blk, AF.Exp, bias=mx[:], accum_out=sm[:])
        rc = sb_sm.tile([P, 1], F32, tag="rrec", name=f"rrec{k}")
        nc.vector.reciprocal(rc[:], sm[:])
        nc.vector.tensor_scalar_mul(blk, blk, rc[:])

    comb = sb_rt.tile([P, TT * E], F32)
    mask_bf = sb_rt.tile([P, TT * E], BF16)
    for k in range(TT):
        blk = scores[:, k * E:(k + 1) * E]
        blk3 = scores[:, k * E:(k + 1) * E].rearrange("p (g f) -> p g f", f=4)
        gsc = sb_sm.tile([P, G_GRP], F32, tag="gsc", name=f"gsc{k}")
        nc.vector.tensor_reduce(gsc[:], blk3, axis=mybir.AxisListType.X, op=OP.max)
        nc.vector.max(out=tmp8[:], in_=gsc[:])
        nc.vector.memset(tmp8[:, TOPK_G:], 0.0)
        gz = sb_sm.tile([P, G_GRP], F32, tag="gz", name=f"gz{k}")
        nc.vector.match_replace(out=gz[:], in_to_replace=tmp8[:],
                                in_values=gsc[:], imm_value=0.0)
        nc.vector.tensor_tensor(out=gz[:], in0=gsc[:], in1=gz[:], op=OP.subtract)
        nc.vector.tensor_scalar(gz[:], gz[:], 0.0, scalar2=None, op0=OP.is_gt)
        cblk = comb[:, k * E:(k + 1) * E]
        cblk3 = comb[:, k * E:(k + 1) * E].rearrange("p (g f) -> p g f", f=4)
        gz3 = gz[:].rearrange("p (g o) -> p g o", o=1)
        nc.vector.tensor_tensor(out=cblk3, in0=blk3,
                                in1=gz3.to_broadcast([P, G_GRP, 4]), op=OP.mult)
        nc.vector.max(out=tmp8[:], in_=cblk)
        nc.vector.memset(tmp8[:, TOPK:], 0.0)
        zap = sb_sm.tile([P, E], F32, tag="zap", name=f"zap{k}")
        nc.vector.match_replace(out=zap[:], in_to_replace=tmp8[:],
                                in_values=cblk, imm_value=0.0)
        nc.vector.tensor_tensor(out=cblk, in0=cblk, in1=zap[:], op=OP.subtract)
        nc.vector.tensor_scalar_mul(cblk, cblk, SCALE)
        nc.vector.tensor_copy(mask_bf[:, k * E:(k + 1) * E], cblk)
        nc.vector.tensor_scalar(mask_bf[:, k * E:(k + 1) * E],
                                mask_bf[:, k * E:(k + 1) * E],
                                0.0, scalar2=None, op0=OP.is_gt)

    # combT (bf16, scaled weights) + maskT
    combT = sb_rt.tile([E, T], BF16)
    maskT = sb_rt.tile([E, T], BF16)
    for k in range(TT):
        pst = ps_r.tile([P, 512], F32, tag="psr", name=f"trc{k}")
        nc.tensor.transpose(pst[:E, :P], comb[:, k * E:(k + 1) * E], ident[:])
        nc.vector.tensor_copy(combT[:, k * P:(k + 1) * P], pst[:E, :P])
    nc.vector.tensor_scalar(maskT[:], combT[:], 0.0, scalar2=None, op0=OP.is_gt)

    # cumsum over tokens -> slot (bf16 exact: masked pos <= C+1 territory)
    pos_bf = sb_rt.tile([E, T], BF16)
    for n in range(2):
        psc = ps_r.tile([P, T // 2], F32, tag="psr", name=f"psc{n}")
        pcs = psc[:E, :]
        for k in range(TT):
            lk = sb_sm.tile([P, T // 2], BF16, tag="lk")
            nc.gpsimd.affine_select(
                out=lk[:], in_=ones_bf[:], pattern=[[1, T // 2]],
                compare_op=OP.is_ge, fill=0.0,
                base=n * (T // 2) - k * P, channel_multiplier=-1)
            nc.tensor.matmul(pcs, mask_bf[:, k * E:(k + 1) * E], lk[:],
                             start=(k == 0), stop=(k == TT - 1))
        # slot = min((pos - 1 - C) * mask + C, C), in-place on psum
        nc.vector.tensor_scalar(pcs, pcs, float(1 + C), scalar2=None,
                                op0=OP.subtract)
        nc.vector.tensor_tensor(out=pcs, in0=pcs,
                                in1=maskT[:, n * 512:(n + 1) * 512], op=OP.mult)
        nc.vector.tensor_scalar(pcs, pcs, float(C), scalar2=None, op0=OP.add)
        nc.vector.tensor_scalar_min(pcs, pcs, float(C))
        nc.vector.tensor_copy(pos_bf[:, n * 512:(n + 1) * 512], pcs)

    # ---- shared expert MM1: act_sT[m, t] for m in 3 gate/up tiles ----
    act_sT = sb.tile([P, SST * T], BF16)
    for n in range(2):
        psg = [ps_s.tile([P, T // 2], F32, tag="pss", name=f"psg{n}{m}")
               for m in range(SST)]
        psu = [ps_s.tile([P, T // 2], F32, tag="pss", name=f"psu{n}{m}")
               for m in range(SST)]
        for ks in range(4):
            ws1t = sb_ws1.tile([P, 4 * 2 * SSH], BF16, tag="ws1")
            nc.scalar.dma_start(
                out=ws1t[:].rearrange("p (c m) -> p c m", c=4),
                in_=ws1_d[:, 4 * ks:4 * ks + 4, :])
            for kk in range(4):
                k = 4 * ks + kk
                xbf = sb_xbf.tile([P, T // 2], BF16, tag="xbf")
                nc.sync.dma_start(out=xbf[:],
                                  in_=xtbf_d[:, k, n * 512:(n + 1) * 512])
                for m in range(SST):
                    nc.tensor.matmul(
                        psg[m][:], ws1t[:, kk * 768 + m * P:kk * 768 + (m + 1) * P],
                        xbf[:], start=(k == 0), stop=(k == HC - 1))
                    nc.tensor.matmul(
                        psu[m][:],
                        ws1t[:, kk * 768 + SSH + m * P:kk * 768 + SSH + (m + 1) * P],
                        xbf[:], start=(k == 0), stop=(k == HC - 1))
        for m in range(SST):
            gsil = sb_sm.tile([P, T // 2], F32, tag="gsil", name=f"gsil{n}{m}")
            nc.scalar.activation(gsil[:], psg[m][:], AF.Sigmoid)
            nc.vector.tensor_tensor(out=gsil[:], in0=gsil[:], in1=psg[m][:],
                                    op=OP.mult)
            nc.vector.tensor_tensor(
                out=act_sT[:, m * T + n * 512:m * T + (n + 1) * 512],
                in0=gsil[:], in1=psu[m][:], op=OP.mult)

    # ---- per-expert prep: slot row, combine row, gmat, slot->token, gather ----
    gmat = sb.tile([P, EL * 2 * T], BF16)       # [c-half, (e, half, t)]
    xets = []                                   # per-expert [h-chunk, C] bf16
    stoks = []
    for e in range(EL):
        sel128 = sb_sm.tile([E, P], BF16, tag="sel128", name=f"sel{e}")
        nc.vector.tensor_copy(sel128[:], sel_sb[:, e:e + 1].to_broadcast([E, P]))
        srow = sb_sm.tile([P, T], BF16, tag="srow", name=f"srow{e}")
        crow = sb_sm.tile([P, T], BF16, tag="crow", name=f"crow{e}")
        for src, dst in ((pos_bf, srow), (combT, crow)):
            for nn in range(2):
                psb = ps_r.tile([P, 512], F32, tag="psr",
                                name=f"bc_{e}_{dst.name}_{nn}")
                nc.tensor.matmul(psb[:], sel128[:],
                                 src[:, nn * 512:(nn + 1) * 512],
                                 start=True, stop=True)
                nc.vector.tensor_copy(dst[:, nn * 512:(nn + 1) * 512], psb[:])
        # gmat[e]: (slot(t) == 128*half + p) * w(t)
        for half in range(2):
            gslc = gmat[:, (e * 2 + half) * T:(e * 2 + half + 1) * T]
            nc.vector.tensor_tensor(
                out=gslc, in0=iota_half[:, half:half + 1].to_broadcast([P, T]),
                in1=srow[:], op=OP.is_equal)
            nc.vector.tensor_tensor(out=gslc, in0=gslc, in1=crow[:], op=OP.mult)
        # slotcol[p, k] = slot(128k + p)
        slotcol = sb_sm.tile([P, TT], F32, tag="slotcol", name=f"slc{e}")
        for k in range(TT):
            pst = ps_r.tile([P, 512], F32, tag="psr", name=f"sc_{e}_{k}")
            pstb = pst[:].bitcast(BF16)
            nc.tensor.transpose(pstb[:, :P], srow[:, k * P:(k + 1) * P], identb[:])
            nc.vector.tensor_copy(slotcol[:, k:k + 1], pstb[:, 0:1])
        # slot_tokens[c] = sum_t (slot[t] == c) * t (exact fp32), as a row
        # [1, 256], then PE-transposed per half to [128, 1] gather offsets
        stok = sb.tile([P, 2], I32, name=f"stok{e}")
        stokrow = sb_sm.tile([1, C], F32, tag="stokrow", name=f"srw{e}")
        pss = ps_r.tile([P, 512], F32, tag="psr", name=f"st_{e}")
        for k in range(TT):
            petk = sb_sm.tile([P, C], F32, tag="petk")
            nc.vector.tensor_tensor(
                out=petk[:], in0=slotcol[:, k:k + 1].to_broadcast([P, C]),
                in1=iota_c_row[:], op=OP.is_equal)
            nc.tensor.matmul(pss[:1, :C], tok_iota[:, k:k + 1], petk[:],
                             start=(k == 0), stop=(k == TT - 1))
        nc.vector.tensor_copy(stokrow[:], pss[:1, :C])
        for half in range(2):
            pst = ps_r.tile([P, 512], F32, tag="psr", name=f"stt_{e}_{half}")
            nc.tensor.transpose(pst[:, :1],
                                stokrow[:, half * P:(half + 1) * P],
                                ident[:1, :1])
            nc.vector.tensor_copy(stok[:, half:half + 1], pst[:, :1])
        stoks.append(stok)
        # gather token rows (bf16) and transpose to [h-chunk, C]
        xet = sb_xet.tile([P, HC * C], BF16, tag="xet", name=f"xet{e}")
        for half in range(2):
            xe = sb_xe.tile([P, H], BF16, tag="xe")
            nc.gpsimd.indirect_dma_start(
                out=xe[:], out_offset=None, in_=xrow_d[:, :],
                in_offset=bass.IndirectOffsetOnAxis(
                    ap=stok[:, half:half + 1], axis=0))
            for hc in range(HC):
                pst = ps_r.tile([P, 512], F32, tag="psr",
                                name=f"xt_{e}_{half}_{hc}")
                pstb = pst[:].bitcast(BF16)
                nc.tensor.transpose(pstb[:, :P], xe[:, hc * P:(hc + 1) * P],
                                    identb[:])
                nc.vector.tensor_copy(
                    xet[:, hc * C + half * P:hc * C + half * P + P], pstb[:, :P])
        xets.append(xet)

    # ---- experts: MM1 (m-tile pairs) + silu*up ----
    acts = []
    for e in range(EL):
        xet = xets[e]
        # gate_e has a 12th (garbage) tile so pair 5's full-tile ops stay
        # whole-bank: partial DVE reads of a bank the PE is still writing
        # are a HW fault, full-tile reads depend on both m-tiles' groups.
        gate_e = sb.tile([P, (FT + 1) * C], BF16, tag="gate", name=f"gate{e}")
        act_e = sb_act.tile([P, FT * C], BF16, tag="act", name=f"act{e}")
        for pr in range(FT):                    # pair pr = m-tiles {2pr, 2pr+1}
            psq = ps_s.tile([P, 2 * C], F32, tag="pss", name=f"mm1_{e}_{pr}")
            w1t = sb_w1.tile([P, HC * 2 * P], BF16, tag="w1")
            nc.scalar.dma_start(
                out=w1t[:].rearrange("p (c f) -> p c f", c=HC),
                in_=w1_d[:, e * FT + pr, :, :])
            for j in range(2):
                for k in range(HC):
                    nc.tensor.matmul(
                        psq[:, j * C:(j + 1) * C],
                        w1t[:, k * 2 * P + j * P:k * 2 * P + (j + 1) * P],
                        xet[:, k * C:(k + 1) * C],
                        start=(k == 0), stop=(k == HC - 1))
            m0 = 2 * pr
            if m0 < FT:     # gate-gate pair (pair 5: m10 gate + m11 garbage)
                sgt = sb_sm.tile([P, 2 * C], F32, tag="sgt", name=f"sgt_{e}_{pr}")
                nc.scalar.activation(sgt[:], psq[:], AF.Sigmoid)
                nc.vector.tensor_tensor(out=gate_e[:, m0 * C:(m0 + 2) * C],
                                        in0=sgt[:], in1=psq[:], op=OP.mult)
                if m0 + 1 == FT:  # pair 5: m11 is really up tile mm=0
                    nc.vector.tensor_tensor(
                        out=act_e[:, 0:C], in0=gate_e[:, 0:C],
                        in1=psq[:, C:2 * C], op=OP.mult)
            else:           # up-up pair: mm = 2pr - 11, 2pr - 10
                mm = m0 - FT
                nc.vector.tensor_tensor(
                    out=act_e[:, mm * C:(mm + 2) * C],
                    in0=gate_e[:, mm * C:(mm + 2) * C],
                    in1=psq[:], op=OP.mult)
        acts.append(act_e)
    # ---- MM2 + shared MM2 + combine, n-outer so the 4 ReduceScatters
    # overlap compute; out-DMAs ride the vector queue so RS completion
    # never blocks later acc stores (sync) or cc triggers (gpsimd) ----
    ws2_sb = sb.tile([P, SST * H], BF16)
    nc.scalar.dma_start(out=ws2_sb[:], in_=ws2_d[:, :, :])
    for n in range(4):
        y_n = sb_yn.tile([P, EL * 2 * 512], BF16, tag="yn", name=f"yn{n}")
        for e in range(EL):
            act_e = acts[e]
            psy = [ps_s.tile([P, 512], F32, tag="pss", name=f"y_{e}_{n}_{mc}")
                   for mc in range(2)]
            for fs in range(2):
                fn = 6 if fs == 0 else 5
                w2t = sb_w2.tile([P, 6 * 512], BF16, tag="w2")
                nc.scalar.dma_start(
                    out=w2t[:, :fn * 512].rearrange("p (c h) -> p c h", c=fn),
                    in_=w2_d[:, e, 6 * fs:6 * fs + fn, n * 512:(n + 1) * 512])
                for kk in range(fn):
                    kf = 6 * fs + kk
                    for mc in range(2):
                        nc.tensor.matmul(
                            psy[mc][:],
                            act_e[:, kf * C + mc * P:kf * C + (mc + 1) * P],
                            w2t[:, kk * 512:(kk + 1) * 512],
                            start=(kf == 0), stop=(kf == FT - 1))
            for mc in range(2):
                nc.scalar.activation(
                    y_n[:, (e * 2 + mc) * 512:(e * 2 + mc + 1) * 512],
                    psy[mc][:], AF.Copy)
        for mt in range(TT):
            pso = ps_r.tile([P, 512], F32, tag="psr", name=f"o_{n}_{mt}")
            for kf in range(SST):
                nc.tensor.matmul(
                    pso[:], act_sT[:, kf * T + mt * P:kf * T + (mt + 1) * P],
                    ws2_sb[:, kf * H + n * 512:kf * H + (n + 1) * 512],
                    start=(kf == 0), stop=False)
            for e in range(EL):
                for half in range(2):
                    nc.tensor.matmul(
                        pso[:],
                        gmat[:, (e * 2 + half) * T + mt * P:
                             (e * 2 + half) * T + (mt + 1) * P],
                        y_n[:, (e * 2 + half) * 512:(e * 2 + half + 1) * 512],
                        start=False, stop=(e == EL - 1 and half == 1))
            stg = sb_st.tile([P, 512], BF16, tag="stg", name=f"stg_{n}_{mt}")
            nc.vector.tensor_copy(stg[:], pso[:])
            nc.sync.dma_start(out=acc_ds[n][mt * P:(mt + 1) * P, :], in_=stg[:])
        nc.gpsimd.collective_compute(
            "ReduceScatter", OP.add,
            replica_groups=[list(range(NCORES))],
            ins=[acc_ds[n][:, :]], outs=[rs_ds[n][:, :]])
        nc.gpsimd.dma_start(out=out_d[:, n * 512:(n + 1) * 512],
                            in_=rs_ds[n][:, :])
    ctx.close()


# ---------------- host side ----------------
_CACHED = {}


def _get_program():
    if "nc" not in _CACHED:
        _CACHED["nc"] = build_program()
    return _CACHED["nc"]


def _tile_pc(a, nchunk):
    """[nchunk*128, M] -> [128, nchunk, M] partition-major tiling."""
    r, m = a.shape
    assert r == nchunk * P
    return np.ascontiguousarray(
        a.reshape(nchunk, P, m).transpose(1, 0, 2))


def make_in_maps(hidden_states, w_gate, w1, w2, ws1, ws2):
    bf = ml_dtypes.bfloat16
    x = np.asarray(hidden_states, np.float32)
    xT = np.ascontiguousarray(x.T)                       # [H, T]
    xt32 = _tile_pc(xT, HC)                              # [128, 16, 1024]
    xtbf = xt32.astype(bf)
    xrow = x.astype(bf)                                  # [T, H]
    wg = _tile_pc(np.ascontiguousarray(np.asarray(w_gate, np.float32).T), HC)
    w1 = np.asarray(w1, np.float32)
    w2 = np.asarray(w2, np.float32)
    ws1 = np.asarray(ws1, np.float32)
    ws2 = np.asarray(ws2, np.float32)
    shard = FS // NCORES  # 352
    in_maps = []
    for k in range(NCORES):
        # w1 local pair-major: [p, e*11+pr, k, f'] = w1[e, k*128+p, pr*256+f']
        w1l = w1[k * EL:(k + 1) * EL].reshape(EL, HC, P, FT, 2 * P)
        w1l = np.ascontiguousarray(w1l.transpose(2, 0, 3, 1, 4)).astype(bf)
        w1l = w1l.reshape(P, EL * FT, HC, 2 * P)
        # w2 local: [4, 1408, 2048] -> [128, 4, 11, 2048]
        w2l = w2[k * EL:(k + 1) * EL].reshape(EL, FT, P, H)
        w2l = np.ascontiguousarray(w2l.transpose(2, 0, 1, 3)).astype(bf)
        ws1p = np.zeros((H, 2 * SSH), np.float32)
        ws1p[:, :shard] = ws1[:, k * shard:(k + 1) * shard]
        ws1p[:, SSH:SSH + shard] = ws1[:, FS + k * shard:FS + (k + 1) * shard]
        ws2p = np.zeros((SSH, H), np.float32)
        ws2p[:shard] = ws2[k * shard:(k + 1) * shard]
        sel = np.zeros((E, EL), np.float32)
        for e in range(EL):
            sel[k * EL + e, e] = 1.0
        in_maps.append({
            "xt32": xt32,
            "xtbf": xtbf,
            "xrow": xrow,
            "wg": wg,
            "w1l": w1l,
            "w2l": w2l,
            "ws1l": _tile_pc(ws1p, HC).astype(bf),
            "ws2l": _tile_pc(ws2p, SST).astype(bf),
            "sel": sel.astype(bf),
        })
    return in_maps


def kernel(hidden_states, w_gate, w1, w2, ws1, ws2):
    from concourse.bass_utils import run_bass_kernel_spmd
    nc = _get_program()
    in_maps = make_in_maps(hidden_states, w_gate, w1, w2, ws1, ws2)
    res = run_bass_kernel_spmd(nc, in_maps, list(range(NCORES)))
    shards = [res.results[k]["out"] for k in range(NCORES)]
    return np.concatenate(shards, axis=0).astype(np.float32)


# revision 26
# speedup vs baseline: 1.0066x; 1.0066x over previous
"""DeepseekV2 MoE layer on 8 TRN2 NeuronCores (expert-parallel), v2.

Sharding: w1/w2 4-experts-per-core; router + token activations replicated;
shared expert tensor-parallel along FS (352/core, padded to 384). Routing
(softmax + grouped top-k) on device. Per core: gather each local expert's
<=256 tokens, MLP in bf16, weighted one-hot combine via matmul; plus the
shared-expert slice. Per-H-column-group partial [T, 512] blocks go through
4 pipelined ReduceScatters; each core emits rows [128k, 128(k+1)).

v2 changes vs baseline: all weights host-pre-tiled partition-major so every
DMA is ~1MB with >=1KB descriptors; x pre-cast to bf16 on host; all
non-router matmuls bf16 (combine was fp32r); DMA triggers on sync/scalar
queues (HWDGE) instead of gpsimd; expert prep (gather/transpose) front-
loaded; final combine accumulates shared+4 experts in PSUM (no SBUF acc);
collective split 4x and overlapped with compute.
"""

import numpy as np
import ml_dtypes

import concourse.bass as bass
import concourse.tile as tile
from concourse import bacc, mybir
from concourse.masks import make_identity

# problem shape
T, H = 1024, 2048
E, F = 32, 1408
F2 = 2 * F                      # 2816
G_GRP, TOPK_G, TOPK = 8, 3, 6
FS = 2 * F                      # 2816 shared intermediate
SCALE = 16.0
NCORES = 8
EL = E // NCORES                # 4 experts per core
C = 256                         # per-expert token capacity (max seen ~214)
P = 128
TT = T // P                     # 8 token tiles
HC = H // P                     # 16 h chunks
FT = F // P                     # 11 f tiles
F2T = F2 // P                   # 22
SSH = 384                       # padded shared shard (352 real)
SST = SSH // P                  # 3 shared m tiles

F32 = mybir.dt.float32
BF16 = mybir.dt.bfloat16
I32 = mybir.dt.int32
AF = mybir.ActivationFunctionType
OP = mybir.AluOpType

def build_program():
    nc = bacc.Bacc("TRN2", target_bir_lowering=False, debug=False,
                   num_devices=NCORES)

    xt32_d = nc.dram_tensor("xt32", [P, HC, T], F32, kind="ExternalInput")
    xtbf_d = nc.dram_tensor("xtbf", [P, HC, T], BF16, kind="ExternalInput")
    xrow_d = nc.dram_tensor("xrow", [T, H], BF16, kind="ExternalInput")
    wg_d = nc.dram_tensor("wg", [P, HC, E], F32, kind="ExternalInput")
    # w1 pair-major: [p, e*11+pair, k, 256] = w1[e, k*128+p, pair*256 + f']
    w1_d = nc.dram_tensor("w1l", [P, EL * FT, HC, 2 * P], BF16,
                          kind="ExternalInput")
    w2_d = nc.dram_tensor("w2l", [P, EL, FT, H], BF16, kind="ExternalInput")
    ws1_d = nc.dram_tensor("ws1l", [P, HC, 2 * SSH], BF16, kind="ExternalInput")
    ws2_d = nc.dram_tensor("ws2l", [P, SST, H], BF16, kind="ExternalInput")
    sel_d = nc.dram_tensor("sel", [E, EL], BF16, kind="ExternalInput")
    out_d = nc.dram_tensor("out", [P, H], F32, kind="ExternalOutput")

    acc_ds = [nc.dram_tensor(f"acc{n}", [T, 512], F32) for n in range(4)]
    rs_ds = [nc.dram_tensor(f"rs{n}", [P, 512], F32) for n in range(4)]

    with tile.TileContext(nc) as tc:
        _build(nc, tc, locals())

    nc.compile()
    return nc


def _build(nc, tc, g):
    xt32_d, xtbf_d, xrow_d, wg_d = g["xt32_d"], g["xtbf_d"], g["xrow_d"], g["wg_d"]
    w1_d, w2_d, ws1_d, ws2_d = g["w1_d"], g["w2_d"], g["ws1_d"], g["ws2_d"]
    sel_d, out_d, acc_ds, rs_ds = g["sel_d"], g["out_d"], g["acc_ds"], g["rs_ds"]

    import contextlib
    ctx = contextlib.ExitStack()
    sb = ctx.enter_context(tc.tile_pool(name="sb", bufs=1))
    sb_rt = ctx.enter_context(tc.tile_pool(name="sb_rt", bufs=1))
    sb_sm = ctx.enter_context(tc.tile_pool(name="sb_sm", bufs=2))
    sb_xt = ctx.enter_context(tc.tile_pool(name="sb_xt", bufs=2))
    sb_xbf = ctx.enter_context(tc.tile_pool(name="sb_xbf", bufs=3))
    sb_ws1 = ctx.enter_context(tc.tile_pool(name="sb_ws1", bufs=2))
    sb_w1 = ctx.enter_context(tc.tile_pool(name="sb_w1", bufs=2))
    sb_w2 = ctx.enter_context(tc.tile_pool(name="sb_w2", bufs=3))
    sb_xe = ctx.enter_context(tc.tile_pool(name="sb_xe", bufs=2))
    sb_xet = ctx.enter_context(tc.tile_pool(name="sb_xet", bufs=2))
    sb_act = ctx.enter_context(tc.tile_pool(name="sb_act", bufs=4))
    sb_yn = ctx.enter_context(tc.tile_pool(name="sb_yn", bufs=2))
    sb_st = ctx.enter_context(tc.tile_pool(name="sb_st", bufs=2))
    ps_s = ctx.enter_context(tc.tile_pool(name="ps_s", bufs=6, space="PSUM"))
    ps_r = ctx.enter_context(tc.tile_pool(name="ps_r", bufs=2, space="PSUM"))

    # ---- constants ----
    ident = sb.tile([P, P], F32)
    make_identity(nc, ident[:])
    identb = sb.tile([P, P], BF16)
    nc.vector.tensor_copy(identb[:], ident[:])
    iota_c_row_i = sb.tile([P, C], I32)
    nc.gpsimd.iota(iota_c_row_i[:], pattern=[[1, C]], base=0, channel_multiplier=0)
    iota_c_row = sb.tile([P, C], F32)
    nc.vector.tensor_copy(iota_c_row[:], iota_c_row_i[:])
    iota_half_i = sb.tile([P, 2], I32)   # col h: value 128*h + p
    nc.gpsimd.iota(iota_half_i[:], pattern=[[P, 2]], base=0, channel_multiplier=1)
    iota_half = sb.tile([P, 2], BF16)
    nc.vector.tensor_copy(iota_half[:], iota_half_i[:])
    tok_iota_i = sb.tile([P, TT], I32)   # col k: value 128*k + p
    nc.gpsimd.iota(tok_iota_i[:], pattern=[[P, TT]], base=0, channel_multiplier=1)
    tok_iota = sb.tile([P, TT], F32)
    nc.vector.tensor_copy(tok_iota[:], tok_iota_i[:])
    ones_bf = sb.tile([P, T // 2], BF16)
    nc.vector.memset(ones_bf[:], 1.0)

    wg_sb = sb.tile([P, HC * E], F32)
    nc.scalar.dma_start(out=wg_sb[:], in_=wg_d[:, :, :])
    sel_sb = sb.tile([E, EL], BF16)
    nc.scalar.dma_start(out=sel_sb[:], in_=sel_d[:, :])

    # ---- router: logT[e, t] = wg.T @ x, fp32 (topk must match fp32 order) ----
    logT = sb_rt.tile([E, T], F32)
    psl = [ps_r.tile([P, T // 2], F32, tag="psr", name=f"psl{n}") for n in range(2)]
    for k in range(HC):
        xt = sb_xt.tile([P, T], F32, tag="xt")
        nc.sync.dma_start(out=xt[:], in_=xt32_d[:, k, :])
        for n in range(2):
            nc.tensor.matmul(psl[n][:E, :], wg_sb[:, k * E:(k + 1) * E],
                             xt[:, n * 512:(n + 1) * 512],
                             start=(k == 0), stop=(k == HC - 1))
    for n in range(2):
        nc.vector.tensor_copy(logT[:, n * 512:(n + 1) * 512], psl[n][:E, :])

    # ---- routing math on [P, TT*E] ----
    scores = sb_rt.tile([P, TT * E], F32)
    for k in range(TT):
        pst = ps_r.tile([P, 512], F32, tag="psr", name=f"tr{k}")
        nc.tensor.transpose(pst[:, :E], logT[:, k * P:(k + 1) * P], ident[:E, :E])
        nc.vector.tensor_copy(scores[:, k * E:(k + 1) * E], pst[:, :E])

    tmp8 = sb_rt.tile([P, 8], F32)
    for k in range(TT):
        blk = scores[:, k * E:(k + 1) * E]
        mx = sb_sm.tile([P, 1], F32, tag="rmax", name=f"rmax{k}")
        nc.vector.tensor_reduce(mx[:], blk, axis=mybir.AxisListType.X,
                                op=OP.max, negate=True)
        sm = sb_sm.tile([P, 1], F32, tag="rsum", name=f"rsum{k}")
        nc.scalar.activation(blk, blk, AF.Exp, bias=mx[:], accum_out=sm[:])
        rc = sb_sm.tile([P, 1], F32, tag="rrec", name=f"rrec{k}")
        nc.vector.reciprocal(rc[:], sm[:])
        nc.vector.tensor_scalar_mul(blk, blk, rc[:])

    comb = sb_rt.tile([P, TT * E], F32)
    mask_bf = sb_rt.tile([P, TT * E], BF16)
    for k in range(TT):
        blk = scores[:, k * E:(k + 1) * E]
        blk3 = scores[:, k * E:(k + 1) * E].rearrange("p (g f) -> p g f", f=4)
        gsc = sb_sm.tile([P, G_GRP], F32, tag="gsc", name=f"gsc{k}")
        nc.vector.tensor_reduce(gsc[:], blk3, axis=mybir.AxisListType.X, op=OP.max)
        nc.vector.max(out=tmp8[:], in_=gsc[:])
        nc.vector.memset(tmp8[:, TOPK_G:], 0.0)
        gz = sb_sm.tile([P, G_GRP], F32, tag="gz", name=f"gz{k}")
        nc.vector.match_replace(out=gz[:], in_to_replace=tmp8[:],
                                in_values=gsc[:], imm_value=0.0)
        nc.vector.tensor_tensor(out=gz[:], in0=gsc[:], in1=gz[:], op=OP.subtract)
        nc.vector.tensor_scalar(gz[:], gz[:], 0.0, scalar2=None, op0=OP.is_gt)
        cblk = comb[:, k * E:(k + 1) * E]
        cblk3 = comb[:, k * E:(k + 1) * E].rearrange("p (g f) -> p g f", f=4)
        gz3 = gz[:].rearrange("p (g o) -> p g o", o=1)
        nc.vector.tensor_tensor(out=cblk3, in0=blk3,
                                in1=gz3.to_broadcast([P, G_GRP, 4]), op=OP.mult)
        nc.vector.max(out=tmp8[:], in_=cblk)
        nc.vector.memset(tmp8[:, TOPK:], 0.0)
        zap = sb_sm.tile([P, E], F32, tag="zap", name=f"zap{k}")
        nc.vector.match_replace(out=zap[:], in_to_replace=tmp8[:],
                                in_values=cblk, imm_value=0.0)
        nc.vector.tensor_tensor(out=cblk, in0=cblk, in1=zap[:], op=OP.subtract)
        nc.vector.tensor_scalar_mul(cblk, cblk, SCALE)
        nc.vector.tensor_copy(mask_bf[:, k * E:(k + 1) * E], cblk)
        nc.vector.tensor_scalar(mask_bf[:, k * E:(k + 1) * E],
                                mask_bf[:, k * E:(k + 1) * E],
                                0.0, scalar2=None, op0=OP.is_gt)

    # combT (bf16, scaled weights) + maskT
    combT = sb_rt.tile([E, T], BF16)
    maskT = sb_rt.tile([E, T], BF16)
    for k in range(TT):
        pst = ps_r.tile([P, 512], F32, tag="psr", name=f"trc{k}")
        nc.tensor.transpose(pst[:E, :P], comb[:, k * E:(k + 1) * E], ident[:])
        nc.vector.tensor_copy(combT[:, k * P:(k + 1) * P], pst[:E, :P])
    nc.vector.tensor_scalar(maskT[:], combT[:], 0.0, scalar2=None, op0=OP.is_gt)

    # cumsum over tokens -> slot (bf16 exact: masked pos <= C+1 territory)
    pos_bf = sb_rt.tile([E, T], BF16)
    for n in range(2):
        psc = ps_r.tile([P, T // 2], F32, tag="psr", name=f"psc{n}")
        pcs = psc[:E, :]
        for k in range(TT):
            lk = sb_sm.tile([P, T // 2], BF16, tag="lk")
            nc.gpsimd.affine_select(
                out=lk[:], in_=ones_bf[:], pattern=[[1, T // 2]],
                compare_op=OP.is_ge, fill=0.0,
                base=n * (T // 2) - k * P, channel_multiplier=-1)
            nc.tensor.matmul(pcs, mask_bf[:, k * E:(k + 1) * E], lk[:],
                             start=(k == 0), stop=(k == TT - 1))
        # slot = min((pos - 1 - C) * mask + C, C), in-place on psum
        nc.vector.tensor_scalar(pcs, pcs, float(1 + C), scalar2=None,
                                op0=OP.subtract)
        nc.vector.tensor_tensor(out=pcs, in0=pcs,
                                in1=maskT[:, n * 512:(n + 1) * 512], op=OP.mult)
        nc.vector.tensor_scalar(pcs, pcs, float(C), scalar2=None, op0=OP.add)
        nc.vector.tensor_scalar_min(pcs, pcs, float(C))
        nc.vector.tensor_copy(pos_bf[:, n * 512:(n + 1) * 512], pcs)

    # ---- shared expert MM1: act_sT[m, t] for m in 3 gate/up tiles ----
    act_sT = sb.tile([P, SST * T], BF16)
    for n in range(2):
        psg = [ps_s.tile([P, T // 2], F32, tag="pss", name=f"psg{n}{m}")
               for m in range(SST)]
        psu = [ps_s.tile([P, T // 2], F32, tag="pss", name=f"psu{n}{m}")
               for m in range(SST)]
        for ks in range(4):
            ws1t = sb_ws1.tile([P, 4 * 2 * SSH], BF16, tag="ws1")
            nc.scalar.dma_start(
                out=ws1t[:].rearrange("p (c m) -> p c m", c=4),
                in_=ws1_d[:, 4 * ks:4 * ks + 4, :])
            for kk in range(4):
                k = 4 * ks + kk
                xbf = sb_xbf.tile([P, T // 2], BF16, tag="xbf")
                nc.sync.dma_start(out=xbf[:],
                                  in_=xtbf_d[:, k, n * 512:(n + 1) * 512])
                for m in range(SST):
                    nc.tensor.matmul(
                        psg[m][:], ws1t[:, kk * 768 + m * P:kk * 768 + (m + 1) * P],
                        xbf[:], start=(k == 0), stop=(k == HC - 1))
                    nc.tensor.matmul(
                        psu[m][:],
                        ws1t[:, kk * 768 + SSH + m * P:kk * 768 + SSH + (m + 1) * P],
                        xbf[:], start=(k == 0), stop=(k == HC - 1))
        for m in range(SST):
            gsil = sb_sm.tile([P, T // 2], F32, tag="gsil", name=f"gsil{n}{m}")
            nc.scalar.activation(gsil[:], psg[m][:], AF.Sigmoid)
            nc.vector.tensor_tensor(out=gsil[:], in0=gsil[:], in1=psg[m][:],
                                    op=OP.mult)
            nc.vector.tensor_tensor(
                out=act_sT[:, m * T + n * 512:m * T + (n + 1) * 512],
                in0=gsil[:], in1=psu[m][:], op=OP.mult)

    # ---- per-expert prep: slot row, combine row, gmat, slot->token, gather ----
    gmat = sb.tile([P, EL * 2 * T], BF16)       # [c-half, (e, half, t)]
    xets = []                                   # per-expert [h-chunk, C] bf16
    stoks = []
    for e in range(EL):
        sel128 = sb_sm.tile([E, P], BF16, tag="sel128", name=f"sel{e}")
        nc.vector.tensor_copy(sel128[:], sel_sb[:, e:e + 1].to_broadcast([E, P]))
        srow = sb_sm.tile([P, T], BF16, tag="srow", name=f"srow{e}")
        crow = sb_sm.tile([P, T], BF16, tag="crow", name=f"crow{e}")
        for src, dst in ((pos_bf, srow), (combT, crow)):
            for nn in range(2):
                psb = ps_r.tile([P, 512], F32, tag="psr",
                                name=f"bc_{e}_{dst.name}_{nn}")
                nc.tensor.matmul(psb[:], sel128[:],
                                 src[:, nn * 512:(nn + 1) * 512],
                                 start=True, stop=True)
                nc.vector.tensor_copy(dst[:, nn * 512:(nn + 1) * 512], psb[:])
        # gmat[e]: (slot(t) == 128*half + p) * w(t)
        for half in range(2):
            gslc = gmat[:, (e * 2 + half) * T:(e * 2 + half + 1) * T]
            nc.vector.tensor_tensor(
                out=gslc, in0=iota_half[:, half:half + 1].to_broadcast([P, T]),
                in1=srow[:], op=OP.is_equal)
            nc.vector.tensor_tensor(out=gslc, in0=gslc, in1=crow[:], op=OP.mult)
        # slotcol[p, k] = slot(128k + p)
        slotcol = sb_sm.tile([P, TT], F32, tag="slotcol", name=f"slc{e}")
        for k in range(TT):
            pst = ps_r.tile([P, 512], F32, tag="psr", name=f"sc_{e}_{k}")
            pstb = pst[:].bitcast(BF16)
            nc.tensor.transpose(pstb[:, :P], srow[:, k * P:(k + 1) * P], identb[:])
            nc.vector.tensor_copy(slotcol[:, k:k + 1], pstb[:, 0:1])
        # slot_tokens[c] = sum_t (slot[t] == c) * t (exact fp32), as a row
        # [1, 256], then PE-transposed per half to [128, 1] gather offsets
        stok = sb.tile([P, 2], I32, name=f"stok{e}")
        stokrow = sb_sm.tile([1, C], F32, tag="stokrow", name=f"srw{e}")
        pss = ps_r.tile([P, 512], F32, tag="psr", name=f"st_{e}")
        for k in range(TT):
            petk = sb_sm.tile([P, C], F32, tag="petk")
            nc.vector.tensor_tensor(
                out=petk[:], in0=slotcol[:, k:k + 1].to_broadcast([P, C]),
                in1=iota_c_row[:], op=OP.is_equal)
            nc.tensor.matmul(pss[:1, :C], tok_iota[:, k:k + 1], petk[:],
                             start=(k == 0), stop=(k == TT - 1))
        nc.vector.tensor_copy(stokrow[:], pss[:1, :C])
        for half in range(2):
            pst = ps_r.tile([P, 512], F32, tag="psr", name=f"stt_{e}_{half}")
            nc.tensor.transpose(pst[:, :1],
                                stokrow[:, half * P:(half + 1) * P],
                                ident[:1, :1])
            nc.vector.tensor_copy(stok[:, half:half + 1], pst[:, :1])
        stoks.append(stok)
        # gather token rows (bf16) and transpose to [h-chunk, C]
        xet = sb_xet.tile([P, HC * C], BF16, tag="xet", name=f"xet{e}")
        for half in range(2):
            xe = sb_xe.tile([P, H], BF16, tag="xe")
            nc.gpsimd.indirect_dma_start(
                out=xe[:], out_offset=None, in_=xrow_d[:, :],
                in_offset=bass.IndirectOffsetOnAxis(
                    ap=stok[:, half:half + 1], axis=0))
            for hc in range(HC):
                pst = ps_r.tile([P, 512], F32, tag="psr",
                                name=f"xt_{e}_{half}_{hc}")
                pstb = pst[:].bitcast(BF16)
                nc.tensor.transpose(pstb[:, :P], xe[:, hc * P:(hc + 1) * P],
                                    identb[:])
                nc.vector.tensor_copy(
                    xet[:, hc * C + half * P:hc * C + half * P + P], pstb[:, :P])
        xets.append(xet)

    # ---- experts: MM1 (m-tile pairs) + silu*up ----
    acts = []
    for e in range(EL):
        xet = xets[e]
        # gate_e has a 12th (garbage) tile so pair 5's full-tile ops stay
        # whole-bank: partial DVE reads of a bank the PE is still writing
        # are a HW fault, full-tile reads depend on both m-tiles' groups.
        gate_e = sb.tile([P, (FT + 1) * C], BF16, tag="gate", name=f"gate{e}")
        act_e = sb_act.tile([P, FT * C], BF16, tag="act", name=f"act{e}")
        for pr in range(FT):                    # pair pr = m-tiles {2pr, 2pr+1}
            psq = ps_s.tile([P, 2 * C], F32, tag="pss", name=f"mm1_{e}_{pr}")
            w1t = sb_w1.tile([P, HC * 2 * P], BF16, tag="w1")
            nc.scalar.dma_start(
                out=w1t[:].rearrange("p (c f) -> p c f", c=HC),
                in_=w1_d[:, e * FT + pr, :, :])
            for j in range(2):
                for k in range(HC):
                    nc.tensor.matmul(
                        psq[:, j * C:(j + 1) * C],
                        w1t[:, k * 2 * P + j * P:k * 2 * P + (j + 1) * P],
                        xet[:, k * C:(k + 1) * C],
                        start=(k == 0), stop=(k == HC - 1))
            m0 = 2 * pr
            if m0 < FT:     # gate-gate pair (pair 5: m10 gate + m11 garbage)
                sgt = sb_sm.tile([P, 2 * C], F32, tag="sgt", name=f"sgt_{e}_{pr}")
                nc.scalar.activation(sgt[:], psq[:], AF.Sigmoid)
                nc.vector.tensor_tensor(out=gate_e[:, m0 * C:(m0 + 2) * C],
                                        in0=sgt[:], in1=psq[:], op=OP.mult)
                if m0 + 1 == FT:  # pair 5: m11 is really up tile mm=0
                    nc.vector.tensor_tensor(
                        out=act_e[:, 0:C], in0=gate_e[:, 0:C],
                        in1=psq[:, C:2 * C], op=OP.mult)
            else:           # up-up pair: mm = 2pr - 11, 2pr - 10
                mm = m0 - FT
                nc.vector.tensor_tensor(
                    out=act_e[:, mm * C:(mm + 2) * C],
                    in0=gate_e[:, mm * C:(mm + 2) * C],
                    in1=psq[:], op=OP.mult)
        acts.append(act_e)
    # ---- MM2 + shared MM2 + combine, n-outer so the 4 ReduceScatters
    # overlap compute; out-DMAs ride the vector queue so RS completion
    # never blocks later acc stores (sync) or cc triggers (gpsimd) ----
    ws2_sb = sb.tile([P, SST * H], BF16)
    nc.scalar.dma_start(out=ws2_sb[:], in_=ws2_d[:, :, :])
    for n in range(4):
        y_n = sb_yn.tile([P, EL * 2 * 512], BF16, tag="yn", name=f"yn{n}")
        for e in range(EL):
            act_e = acts[e]
            psy = [ps_s.tile([P, 512], F32, tag="pss", name=f"y_{e}_{n}_{mc}")
                   for mc in range(2)]
            for fs in range(2):
                fn = 6 if fs == 0 else 5
                w2t = sb_w2.tile([P, 6 * 512], BF16, tag="w2")
                nc.scalar.dma_start(
                    out=w2t[:, :fn * 512].rearrange("p (c h) -> p c h", c=fn),
                    in_=w2_d[:, e, 6 * fs:6 * fs + fn, n * 512:(n + 1) * 512])
                for kk in range(fn):
                    kf = 6 * fs + kk
                    for mc in range(2):
                        nc.tensor.matmul(
                            psy[mc][:],
                            act_e[:, kf * C + mc * P:kf * C + (mc + 1) * P],
                            w2t[:, kk * 512:(kk + 1) * 512],
                            start=(kf == 0), stop=(kf == FT - 1))
            for mc in range(2):
                nc.scalar.activation(
                    y_n[:, (e * 2 + mc) * 512:(e * 2 + mc + 1) * 512],
                    psy[mc][:], AF.Copy)
        for mt in range(TT):
            pso = ps_r.tile([P, 512], F32, tag="psr", name=f"o_{n}_{mt}")
            for kf in range(SST):
                nc.tensor.matmul(
                    pso[:], act_sT[:, kf * T + mt * P:kf * T + (mt + 1) * P],
                    ws2_sb[:, kf * H + n * 512:kf * H + (n + 1) * 512],
                    start=(kf == 0), stop=False)
            for e in range(EL):
                for half in range(2):
                    nc.tensor.matmul(
                        pso[:],
                        gmat[:, (e * 2 + half) * T + mt * P:
                             (e * 2 + half) * T + (mt + 1) * P],
                        y_n[:, (e * 2 + half) * 512:(e * 2 + half + 1) * 512],
                        start=False, stop=(e == EL - 1 and half == 1))
            stg = sb_st.tile([P, 512], F32, tag="stg", name=f"stg_{n}_{mt}")
            nc.vector.tensor_copy(stg[:], pso[:])
            nc.sync.dma_start(out=acc_ds[n][mt * P:(mt + 1) * P, :], in_=stg[:])
        nc.gpsimd.collective_compute(
            "ReduceScatter", OP.add,
            replica_groups=[list(range(NCORES))],
            ins=[acc_ds[n][:, :]], outs=[rs_ds[n][:, :]])
        nc.gpsimd.dma_start(out=out_d[:, n * 512:(n + 1) * 512],
                            in_=rs_ds[n][:, :])
    ctx.close()


# ---------------- host side ----------------
_CACHED = {}


def _get_program():
    if "nc" not in _CACHED:
        _CACHED["nc"] = build_program()
    return _CACHED["nc"]


def _tile_pc(a, nchunk):
    """[nchunk*128, M] -> [128, nchunk, M] partition-major tiling."""
    r, m = a.shape
    assert r == nchunk * P
    return np.ascontiguousarray(
        a.reshape(nchunk, P, m).transpose(1, 0, 2))


def make_in_maps(hidden_states, w_gate, w1, w2, ws1, ws2):
    bf = ml_dtypes.bfloat16
    x = np.asarray(hidden_states, np.float32)
    xT = np.ascontiguousarray(x.T)                       # [H, T]
    xt32 = _tile_pc(xT, HC)                              # [128, 16, 1024]
    xtbf = xt32.astype(bf)
    xrow = x.astype(bf)                                  # [T, H]
    wg = _tile_pc(np.ascontiguousarray(np.asarray(w_gate, np.float32).T), HC)
    w1 = np.asarray(w1, np.float32)
    w2 = np.asarray(w2, np.float32)
    ws1 = np.asarray(ws1, np.float32)
    ws2 = np.asarray(ws2, np.float32)
    shard = FS // NCORES  # 352
    in_maps = []
    for k in range(NCORES):
        # w1 local pair-major: [p, e*11+pr, k, f'] = w1[e, k*128+p, pr*256+f']
        w1l = w1[k * EL:(k + 1) * EL].reshape(EL, HC, P, FT, 2 * P)
        w1l = np.ascontiguousarray(w1l.transpose(2, 0, 3, 1, 4)).astype(bf)
        w1l = w1l.reshape(P, EL * FT, HC, 2 * P)
        # w2 local: [4, 1408, 2048] -> [128, 4, 11, 2048]
        w2l = w2[k * EL:(k + 1) * EL].reshape(EL, FT, P, H)
        w2l = np.ascontiguousarray(w2l.transpose(2, 0, 1, 3)).astype(bf)
        ws1p = np.zeros((H, 2 * SSH), np.float32)
        ws1p[:, :shard] = ws1[:, k * shard:(k + 1) * shard]
        ws1p[:, SSH:SSH + shard] = ws1[:, FS + k * shard:FS + (k + 1) * shard]
        ws2p = np.zeros((SSH, H), np.float32)
        ws2p[:shard] = ws2[k * shard:(k + 1) * shard]
        sel = np.zeros((E, EL), np.float32)
        for e in range(EL):
            sel[k * EL + e, e] = 1.0
        in_maps.append({
            "xt32": xt32,
            "xtbf": xtbf,
            "xrow": xrow,
            "wg": wg,
            "w1l": w1l,
            "w2l": w2l,
            "ws1l": _tile_pc(ws1p, HC).astype(bf),
            "ws2l": _tile_pc(ws2p, SST).astype(bf),
            "sel": sel.astype(bf),
        })
    return in_maps


def kernel(hidden_states, w_gate, w1, w2, ws1, ws2):
    from concourse.bass_utils import run_bass_kernel_spmd
    nc = _get_program()
    in_maps = make_in_maps(hidden_states, w_gate, w1, w2, ws1, ws2)
    res = run_bass_kernel_spmd(nc, in_maps, list(range(NCORES)))
    shards = [res.results[k]["out"] for k in range(NCORES)]
    return np.concatenate(shards, axis=0).astype(np.float32)
